# revision 7
# baseline (speedup 1.0000x reference)
"""Trainium2 Bass kernel for nn_AwareDecoder segment first/last gather.

Problem: input [16, 2048, 1024] f32, number_mask [16, 2048] int64 with ids in
[0, 512]. For each segment id i in [0, 512): find first/last row-major token
position with that id, gather those rows of the flattened input, concat ->
out [512, 2048] f32.

Strategy (8 NeuronCores, segment-sharded - no collectives):
  core c owns segments [64c, 64c+64). Each core:
    - DMAs the (tiny, 256KB) id array as int32 (lo,hi) pairs; chunk p =
      tokens [256p, 256p+256) lives on partition p,
    - bit-packs per-chunk presence of its 64 segments into 2 int32 words per
      chunk (fused eq-compare on the id high bits + variable left-shift by
      the id low bits, then a bitwise-OR tree over the 256 tokens),
    - decodes first/last chunk per segment: bit-test, position encode, then
      a gpsimd partition_all_reduce(max) which also broadcasts, so the
      argmax chunk one-hot is a single eq-compare (no transposes),
    - gathers the candidate chunks' ids ON-CHIP with a one-hot PE matmul
      that also emits the selected chunk index as an extra column, then
      finds the exact within-chunk position with one fused
      scalar_tensor_tensor + max-reduce,
    - turns (chunk, pos) into global row indices and pulls its 64 first +
      64 last rows (4KB each) straight from HBM with two pipelined hardware
      indirect DMAs (reads only 512KB of the 128MB input), overlapping the
      output writeback of the first half with the second gather.
Host concatenates the 8 slices.
"""
import numpy as np

import concourse.bass as bass
import concourse.tile as tile
from concourse import bacc, bass_isa, mybir
from concourse import bass_utils

P = 128            # partitions / token chunks
L = 32768          # B*S tokens
H = 1024           # hidden
NSEG = 512         # segments
NCORES = 8
SEG_PER_CORE = NSEG // NCORES            # 64
TOK = L // P                             # 256 tokens per chunk
F32 = mybir.dt.float32
F16 = mybir.dt.float16
I32 = mybir.dt.int32

# cf16 layout (f16): [0:256] refine pos encode, [256] min-chunk encode
# (128-p), [257] max-chunk encode (p+1), [258] chunk index p, [259] pad
CFH_W = 260
# cf32 layout (f32): [0] side sign, [1] global decode const, [2] pad,
# [3] segment id, [4] word id 0, [5] word id 1
CFS_W = 6


def build_nc():
    nc = bacc.Bacc("TRN2", target_bir_lowering=False, debug=False)

    x = nc.dram_tensor("x", [L, H], F32, kind="ExternalInput")
    idpairs = nc.dram_tensor("idpairs", [P, TOK, 2], I32, kind="ExternalInput")
    cf16 = nc.dram_tensor("cf16", [P, CFH_W], F16, kind="ExternalInput")
    cf32 = nc.dram_tensor("cf32", [P, CFS_W], F32, kind="ExternalInput")
    ci32 = nc.dram_tensor("ci32", [P, SEG_PER_CORE], I32, kind="ExternalInput")
    out = nc.dram_tensor("out", [SEG_PER_CORE, 2 * H], F32, kind="ExternalOutput")

    A = mybir.AluOpType

    with tile.TileContext(nc) as tc:
        with tc.tile_pool(name="sb", bufs=1) as sb, \
             tc.tile_pool(name="ps", bufs=1, space="PSUM") as ps:

            # ---- loads (parallel queues; idpairs split across two) ----
            idp = sb.tile([P, TOK, 2], I32)
            nc.sync.dma_start(idp[:, 0:TOK // 2, :],
                              idpairs.ap()[:, 0:TOK // 2, :])
            nc.scalar.dma_start(idp[:, TOK // 2:TOK, :],
                                idpairs.ap()[:, TOK // 2:TOK, :])
            cfh = sb.tile([P, CFH_W], F16)
            nc.gpsimd.dma_start(cfh[:], cf16.ap())
            cfs = sb.tile([P, CFS_W], F32)
            nc.gpsimd.dma_start(cfs[:], cf32.ap())
            cis = sb.tile([P, SEG_PER_CORE], I32)
            nc.gpsimd.dma_start(cis[:], ci32.ap())
            # f16 ids + chunk-index column for the PE gather (scalar engine,
            # off the DVE critical path)
            idsf = sb.tile([P, TOK], F16)
            nc.scalar.copy(idsf[:], idp[:, :, 0])

            # ---- main pass: bit-packed presence per (chunk, segment) ----
            lo5 = sb.tile([P, TOK], I32)
            nc.vector.tensor_scalar(lo5[:], idp[:, :, 0], 31, None,
                                    op0=A.bitwise_and)
            hi4 = sb.tile([P, TOK], I32)
            nc.vector.tensor_scalar(hi4[:], idp[:, :, 0], 5, None,
                                    op0=A.arith_shift_right)
            cand = sb.tile([P, 2, TOK], I32)
            eq0 = sb.tile([P, TOK], I32)
            nc.vector.tensor_scalar(eq0[:], hi4[:], cfs[:, 4:5], None,
                                    op0=A.is_equal)
            nc.vector.tensor_tensor(out=cand[:, 0], in0=eq0[:], in1=lo5[:],
                                    op=A.logical_shift_left)
            eq1 = sb.tile([P, TOK], I32)
            nc.vector.tensor_scalar(eq1[:], hi4[:], cfs[:, 5:6], None,
                                    op0=A.is_equal)
            nc.vector.tensor_tensor(out=cand[:, 1], in0=eq1[:], in1=lo5[:],
                                    op=A.logical_shift_left)
            # bitwise-OR tree over the token axis: 256 -> 1 per word
            lv = cand
            width = TOK
            while width > 1:
                half = width // 2
                nxt = sb.tile([P, 2, half], I32, tag=f"or{half}")
                nc.vector.tensor_tensor(out=nxt[:], in0=lv[:, :, 0:half],
                                        in1=lv[:, :, half:width],
                                        op=A.bitwise_or)
                lv = nxt
                width = half
            words = lv                                    # [P, 2, 1]

            # ---- decode: first/last chunk per segment ----
            bits_in = words[:, :, 0].unsqueeze(2).broadcast_to([P, 2, 32])
            cis_v = cis[:, 0:SEG_PER_CORE].rearrange("p (a b) -> p a b", a=2)
            andm = sb.tile([P, 2, 32], I32)
            nc.vector.tensor_tensor(out=andm[:], in0=bits_in, in1=cis_v,
                                    op=A.bitwise_and)
            andf = andm[:].rearrange("p a b -> p (a b)")
            enc = sb.tile([P, P], F16)
            nc.vector.scalar_tensor_tensor(
                out=enc[:, 0:SEG_PER_CORE], in0=andf, scalar=0,
                in1=cfh[:, 256:257].broadcast_to([P, SEG_PER_CORE]),
                op0=A.not_equal, op1=A.mult)
            nc.vector.scalar_tensor_tensor(
                out=enc[:, SEG_PER_CORE:P], in0=andf, scalar=0,
                in1=cfh[:, 257:258].broadcast_to([P, SEG_PER_CORE]),
                op0=A.not_equal, op1=A.mult)
            # max over chunks, replicated to every partition (gpsimd)
            valbc = sb.tile([P, P], F16)
            nc.gpsimd.partition_all_reduce(valbc[:], enc[:], channels=P,
                                           reduce_op=bass_isa.ReduceOp.max)
            # argmax chunk one-hot (encodes are distinct per chunk)
            onehot = sb.tile([P, P], F16)
            nc.vector.tensor_tensor(out=onehot[:], in0=enc[:], in1=valbc[:],
                                    op=A.is_equal)

            # ---- on-chip gather of candidate chunks' ids via PE ----
            grows = ps.tile([P, TOK], F32)
            nc.tensor.matmul(grows[:], onehot[:], idsf[:],
                             start=True, stop=True)
            gsel = ps.tile([P, 1], F32)
            nc.tensor.matmul(gsel[:], onehot[:], cfh[:, 258:259],
                             start=True, stop=True)

            # ---- refine: exact within-chunk position ----
            encr = sb.tile([P, TOK], F16)
            nc.vector.scalar_tensor_tensor(
                out=encr[:], in0=grows[:], scalar=cfs[:, 3:4],
                in1=cfh[:, 0:TOK], op0=A.is_equal, op1=A.mult)
            val2 = sb.tile([P, 1], F32)
            nc.vector.tensor_reduce(val2[:], encr[:],
                                    axis=mybir.AxisListType.X,
                                    op=A.max)

            # ---- global row index: clamp(256*chunk + sgn*val2 + cst) ----
            g = sb.tile([P, 1], F32)
            nc.vector.tensor_scalar(g[:], gsel[:], float(TOK), None,
                                    op0=A.mult)
            gt = sb.tile([P, 1], F32)
            nc.vector.scalar_tensor_tensor(
                out=gt[:], in0=val2[:], scalar=cfs[:, 0:1], in1=g[:],
                op0=A.mult, op1=A.add)
            nc.vector.tensor_scalar(gt[:], gt[:], cfs[:, 1:2], None,
                                    op0=A.add)
            nc.vector.tensor_scalar(gt[:], gt[:], float(L - 1), 0.0,
                                    op0=A.min, op1=A.max)
            gi = sb.tile([P, 1], I32)
            nc.vector.tensor_copy(gi[:], gt[:])

            # ---- gather rows, write out ----
            rows = sb.tile([P, H], F32)
            nc.gpsimd.indirect_dma_start(
                out=rows[:], out_offset=None, in_=x.ap(),
                in_offset=bass.IndirectOffsetOnAxis(ap=gi[:, 0:1], axis=0))
            nc.sync.dma_start(out.ap()[:, 0:H], rows[0:SEG_PER_CORE, :])
            nc.scalar.dma_start(out.ap()[:, H:2 * H], rows[SEG_PER_CORE:P, :])

    nc.compile()
    return nc


_NC = None


def _get_nc():
    global _NC
    if _NC is None:
        _NC = build_nc()
    return _NC


def make_in_maps(input, number_mask):
    x = np.ascontiguousarray(np.asarray(input), dtype=np.float32).reshape(L, H)
    nm = np.ascontiguousarray(np.asarray(number_mask))
    if nm.dtype != np.int64:
        nm = nm.astype(np.int64)
    idpairs = nm.reshape(L).view(np.int32).reshape(P, TOK, 2)

    r = np.arange(P)
    side_max = r >= SEG_PER_CORE                  # rows 0-63 min, 64-127 max
    t = np.arange(TOK, dtype=np.float16)
    cf16 = np.zeros((P, CFH_W), dtype=np.float16)
    cf16[:SEG_PER_CORE, 0:TOK] = TOK - t          # refine encode, min side
    cf16[SEG_PER_CORE:, 0:TOK] = t + 1            # refine encode, max side
    cf16[:, 256] = (P - r).astype(np.float16)     # min-chunk encode coeff
    cf16[:, 257] = (r + 1).astype(np.float16)     # max-chunk encode coeff
    cf16[:, 258] = r.astype(np.float16)           # chunk index

    maskbits = np.tile((np.int32(1) << (np.arange(SEG_PER_CORE, dtype=np.int32)
                                        % 32)), (P, 1)).astype(np.int32)

    in_maps = []
    for c in range(NCORES):
        cf32 = np.zeros((P, CFS_W), dtype=np.float32)
        cf32[:, 0] = np.where(side_max, 1.0, -1.0)          # sgn
        cf32[:, 1] = np.where(side_max, -1.0, float(TOK))   # cst2
        cf32[:, 3] = c * SEG_PER_CORE + (r % SEG_PER_CORE)  # segment id
        cf32[:, 4] = 2 * c                                  # word id 0
        cf32[:, 5] = 2 * c + 1                              # word id 1
        in_maps.append({"x": x, "idpairs": idpairs, "cf16": cf16,
                        "cf32": cf32, "ci32": maskbits})
    return in_maps


def kernel(input, number_mask, n, concat, **_):
    assert int(n) == NSEG and int(concat) == 1
    nc = _get_nc()
    in_maps = make_in_maps(input, number_mask)
    res = bass_utils.run_bass_kernel_spmd(nc, in_maps, core_ids=list(range(NCORES)))
    return np.concatenate([res.results[c]["out"] for c in range(NCORES)], axis=0)


# revision 8
# speedup vs baseline: 1.1201x; 1.1201x over previous
"""Trainium2 Bass kernel for nn_AwareDecoder segment first/last gather.

Problem: input [16, 2048, 1024] f32, number_mask [16, 2048] int64 with ids in
[0, 512]. For each segment id i in [0, 512): find first/last row-major token
position with that id, gather those rows of the flattened input, concat ->
out [512, 2048] f32.

Strategy (8 NeuronCores, segment-sharded - no collectives):
  core c owns segments [64c, 64c+64). Each core:
    - DMAs the (tiny, 256KB) id array as int32 (lo,hi) pairs; chunk p =
      tokens [256p, 256p+256) lives on partition p,
    - bit-packs per-chunk presence of its 64 segments into 2 int32 words per
      chunk (fused eq-compare on the id high bits + variable left-shift by
      the id low bits, then a bitwise-OR tree over the 256 tokens),
    - decodes first/last chunk per segment: bit-test, position encode, then
      a gpsimd partition_all_reduce(max) which also broadcasts, so the
      argmax chunk one-hot is a single eq-compare (no transposes),
    - gathers the candidate chunks' ids ON-CHIP with a one-hot PE matmul
      that also emits the selected chunk index as an extra column, then
      finds the exact within-chunk position with one fused
      scalar_tensor_tensor + max-reduce,
    - turns (chunk, pos) into global row indices and pulls its 64 first +
      64 last rows (4KB each) straight from HBM with two pipelined hardware
      indirect DMAs (reads only 512KB of the 128MB input), overlapping the
      output writeback of the first half with the second gather.
Host concatenates the 8 slices.
"""
import numpy as np

import concourse.bass as bass
import concourse.tile as tile
from concourse import bacc, bass_isa, mybir
from concourse import bass_utils

P = 128            # partitions / token chunks
L = 32768          # B*S tokens
H = 1024           # hidden
NSEG = 512         # segments
NCORES = 8
SEG_PER_CORE = NSEG // NCORES            # 64
TOK = L // P                             # 256 tokens per chunk
F32 = mybir.dt.float32
F16 = mybir.dt.float16
I32 = mybir.dt.int32

# cf16 layout (f16): [0:256] refine pos encode, [256] min-chunk encode
# (128-p), [257] max-chunk encode (p+1), [258] chunk index p, [259] pad
CFH_W = 260
# cf32 layout (f32): [0] side sign, [1] global decode const, [2] pad,
# [3] segment id, [4] word id 0, [5] word id 1
CFS_W = 6


def build_nc():
    nc = bacc.Bacc("TRN2", target_bir_lowering=False, debug=False)

    x = nc.dram_tensor("x", [L, H], F32, kind="ExternalInput")
    idpairs = nc.dram_tensor("idpairs", [P, TOK, 2], I32, kind="ExternalInput")
    cf16 = nc.dram_tensor("cf16", [P, CFH_W], F16, kind="ExternalInput")
    cf32 = nc.dram_tensor("cf32", [P, CFS_W], F32, kind="ExternalInput")
    ci32 = nc.dram_tensor("ci32", [P, SEG_PER_CORE], I32, kind="ExternalInput")
    out = nc.dram_tensor("out", [SEG_PER_CORE, 2 * H], F32, kind="ExternalOutput")

    A = mybir.AluOpType

    with tile.TileContext(nc) as tc:
        with tc.tile_pool(name="sb", bufs=1) as sb, \
             tc.tile_pool(name="ps", bufs=1, space="PSUM") as ps:

            # ---- loads (parallel queues; idpairs split across two) ----
            idp = sb.tile([P, TOK, 2], I32)
            nc.sync.dma_start(idp[:, 0:TOK // 2, :],
                              idpairs.ap()[:, 0:TOK // 2, :])
            nc.scalar.dma_start(idp[:, TOK // 2:TOK, :],
                                idpairs.ap()[:, TOK // 2:TOK, :])
            cfh = sb.tile([P, CFH_W], F16)
            nc.gpsimd.dma_start(cfh[:], cf16.ap())
            cfs = sb.tile([P, CFS_W], F32)
            nc.gpsimd.dma_start(cfs[:], cf32.ap())
            cis = sb.tile([P, SEG_PER_CORE], I32)
            nc.gpsimd.dma_start(cis[:], ci32.ap())
            # f16 ids + chunk-index column for the PE gather (scalar engine,
            # off the DVE critical path)
            idsf = sb.tile([P, TOK], F16)
            nc.scalar.copy(idsf[:], idp[:, :, 0])
            # warm up the gpsimd partition-reduce microcode while the main
            # pass runs, so the real call below starts without a load stall
            warm = sb.tile([P, 2], F16)
            nc.gpsimd.partition_all_reduce(warm[:], cfh[:, 0:2], channels=P,
                                           reduce_op=bass_isa.ReduceOp.max)

            # ---- main pass: bit-packed presence per (chunk, segment) ----
            lo5 = sb.tile([P, TOK], I32)
            nc.vector.tensor_scalar(lo5[:], idp[:, :, 0], 31, None,
                                    op0=A.bitwise_and)
            hi4 = sb.tile([P, TOK], I32)
            nc.vector.tensor_scalar(hi4[:], idp[:, :, 0], 5, None,
                                    op0=A.arith_shift_right)
            cand = sb.tile([P, 2, TOK], I32)
            eq0 = sb.tile([P, TOK], I32)
            nc.vector.tensor_scalar(eq0[:], hi4[:], cfs[:, 4:5], None,
                                    op0=A.is_equal)
            nc.vector.tensor_tensor(out=cand[:, 0], in0=eq0[:], in1=lo5[:],
                                    op=A.logical_shift_left)
            eq1 = sb.tile([P, TOK], I32)
            nc.vector.tensor_scalar(eq1[:], hi4[:], cfs[:, 5:6], None,
                                    op0=A.is_equal)
            nc.vector.tensor_tensor(out=cand[:, 1], in0=eq1[:], in1=lo5[:],
                                    op=A.logical_shift_left)
            # bitwise-OR tree over the token axis: 256 -> 1 per word
            lv = cand
            width = TOK
            while width > 1:
                half = width // 2
                nxt = sb.tile([P, 2, half], I32, tag=f"or{half}")
                nc.vector.tensor_tensor(out=nxt[:], in0=lv[:, :, 0:half],
                                        in1=lv[:, :, half:width],
                                        op=A.bitwise_or)
                lv = nxt
                width = half
            words = lv                                    # [P, 2, 1]

            # ---- decode: first/last chunk per segment ----
            bits_in = words[:, :, 0].unsqueeze(2).broadcast_to([P, 2, 32])
            cis_v = cis[:, 0:SEG_PER_CORE].rearrange("p (a b) -> p a b", a=2)
            andm = sb.tile([P, 2, 32], I32)
            nc.vector.tensor_tensor(out=andm[:], in0=bits_in, in1=cis_v,
                                    op=A.bitwise_and)
            andf = andm[:].rearrange("p a b -> p (a b)")
            enc = sb.tile([P, P], F16)
            nc.vector.scalar_tensor_tensor(
                out=enc[:, 0:SEG_PER_CORE], in0=andf, scalar=0,
                in1=cfh[:, 256:257].broadcast_to([P, SEG_PER_CORE]),
                op0=A.not_equal, op1=A.mult)
            nc.vector.scalar_tensor_tensor(
                out=enc[:, SEG_PER_CORE:P], in0=andf, scalar=0,
                in1=cfh[:, 257:258].broadcast_to([P, SEG_PER_CORE]),
                op0=A.not_equal, op1=A.mult)
            # max over chunks, replicated to every partition (gpsimd)
            valbc = sb.tile([P, P], F16)
            nc.gpsimd.partition_all_reduce(valbc[:], enc[:], channels=P,
                                           reduce_op=bass_isa.ReduceOp.max)
            # argmax chunk one-hot (encodes are distinct per chunk)
            onehot = sb.tile([P, P], F16)
            nc.vector.tensor_tensor(out=onehot[:], in0=enc[:], in1=valbc[:],
                                    op=A.is_equal)

            # ---- on-chip gather of candidate chunks' ids via PE ----
            grows = ps.tile([P, TOK], F32)
            nc.tensor.matmul(grows[:], onehot[:], idsf[:],
                             start=True, stop=True)
            gsel = ps.tile([P, 1], F32)
            nc.tensor.matmul(gsel[:], onehot[:], cfh[:, 258:259],
                             start=True, stop=True)

            # ---- refine: exact within-chunk position ----
            encr = sb.tile([P, TOK], F16)
            nc.vector.scalar_tensor_tensor(
                out=encr[:], in0=grows[:], scalar=cfs[:, 3:4],
                in1=cfh[:, 0:TOK], op0=A.is_equal, op1=A.mult)
            val2 = sb.tile([P, 1], F32)
            nc.vector.tensor_reduce(val2[:], encr[:],
                                    axis=mybir.AxisListType.X,
                                    op=A.max)

            # ---- global row index: clamp(256*chunk + sgn*val2 + cst) ----
            g = sb.tile([P, 1], F32)
            nc.vector.tensor_scalar(g[:], gsel[:], float(TOK), None,
                                    op0=A.mult)
            gt = sb.tile([P, 1], F32)
            nc.vector.scalar_tensor_tensor(
                out=gt[:], in0=val2[:], scalar=cfs[:, 0:1], in1=g[:],
                op0=A.mult, op1=A.add)
            nc.vector.tensor_scalar(gt[:], gt[:], cfs[:, 1:2], None,
                                    op0=A.add)
            nc.vector.tensor_scalar(gt[:], gt[:], float(L - 1), 0.0,
                                    op0=A.min, op1=A.max)
            gi = sb.tile([P, 1], I32)
            nc.vector.tensor_copy(gi[:], gt[:])

            # ---- gather rows, write out ----
            rows = sb.tile([P, H], F32)
            nc.gpsimd.indirect_dma_start(
                out=rows[:], out_offset=None, in_=x.ap(),
                in_offset=bass.IndirectOffsetOnAxis(ap=gi[:, 0:1], axis=0))
            nc.sync.dma_start(out.ap()[:, 0:H], rows[0:SEG_PER_CORE, :])
            nc.scalar.dma_start(out.ap()[:, H:2 * H], rows[SEG_PER_CORE:P, :])

    nc.compile()
    return nc


_NC = None


def _get_nc():
    global _NC
    if _NC is None:
        _NC = build_nc()
    return _NC


def make_in_maps(input, number_mask):
    x = np.ascontiguousarray(np.asarray(input), dtype=np.float32).reshape(L, H)
    nm = np.ascontiguousarray(np.asarray(number_mask))
    if nm.dtype != np.int64:
        nm = nm.astype(np.int64)
    idpairs = nm.reshape(L).view(np.int32).reshape(P, TOK, 2)

    r = np.arange(P)
    side_max = r >= SEG_PER_CORE                  # rows 0-63 min, 64-127 max
    t = np.arange(TOK, dtype=np.float16)
    cf16 = np.zeros((P, CFH_W), dtype=np.float16)
    cf16[:SEG_PER_CORE, 0:TOK] = TOK - t          # refine encode, min side
    cf16[SEG_PER_CORE:, 0:TOK] = t + 1            # refine encode, max side
    cf16[:, 256] = (P - r).astype(np.float16)     # min-chunk encode coeff
    cf16[:, 257] = (r + 1).astype(np.float16)     # max-chunk encode coeff
    cf16[:, 258] = r.astype(np.float16)           # chunk index

    maskbits = np.tile((np.int32(1) << (np.arange(SEG_PER_CORE, dtype=np.int32)
                                        % 32)), (P, 1)).astype(np.int32)

    in_maps = []
    for c in range(NCORES):
        cf32 = np.zeros((P, CFS_W), dtype=np.float32)
        cf32[:, 0] = np.where(side_max, 1.0, -1.0)          # sgn
        cf32[:, 1] = np.where(side_max, -1.0, float(TOK))   # cst2
        cf32[:, 3] = c * SEG_PER_CORE + (r % SEG_PER_CORE)  # segment id
        cf32[:, 4] = 2 * c                                  # word id 0
        cf32[:, 5] = 2 * c + 1                              # word id 1
        in_maps.append({"x": x, "idpairs": idpairs, "cf16": cf16,
                        "cf32": cf32, "ci32": maskbits})
    return in_maps


def kernel(input, number_mask, n, concat, **_):
    assert int(n) == NSEG and int(concat) == 1
    nc = _get_nc()
    in_maps = make_in_maps(input, number_mask)
    res = bass_utils.run_bass_kernel_spmd(nc, in_maps, core_ids=list(range(NCORES)))
    return np.concatenate([res.results[c]["out"] for c in range(NCORES)], axis=0)


# revision 11
# speedup vs baseline: 1.1843x; 1.0573x over previous
"""Trainium2 Bass kernel for nn_AwareDecoder segment first/last gather.

Problem: input [16, 2048, 1024] f32, number_mask [16, 2048] int64 with ids in
[0, 512]. For each segment id i in [0, 512): find first/last row-major token
position with that id, gather those rows of the flattened input, concat ->
out [512, 2048] f32.

Strategy (8 NeuronCores, segment-sharded - no collectives):
  core c owns segments [64c, 64c+64). Each core:
    - DMAs the (tiny, 256KB) id array as int32 (lo,hi) pairs; chunk p =
      tokens [256p, 256p+256) lives on partition p,
    - bit-packs per-chunk presence of its 64 segments into 2 int32 words per
      chunk (fused eq-compare on the id high bits + variable left-shift by
      the id low bits, then a bitwise-OR tree over the 256 tokens),
    - decodes first/last chunk per segment: bit-test, position encode, then
      a gpsimd partition_all_reduce(max) which also broadcasts, so the
      argmax chunk one-hot is a single eq-compare (no transposes),
    - gathers the candidate chunks' ids ON-CHIP with a one-hot PE matmul
      that also emits the selected chunk index as an extra column, then
      finds the exact within-chunk position with one fused
      scalar_tensor_tensor + max-reduce,
    - turns (chunk, pos) into global row indices and pulls its 64 first +
      64 last rows (4KB each) straight from HBM with two pipelined hardware
      indirect DMAs (reads only 512KB of the 128MB input), overlapping the
      output writeback of the first half with the second gather.
Host concatenates the 8 slices.
"""
import numpy as np

import concourse.bass as bass
import concourse.tile as tile
from concourse import bacc, bass_isa, mybir
from concourse import bass_utils

P = 128            # partitions / token chunks
L = 32768          # B*S tokens
H = 1024           # hidden
NSEG = 512         # segments
NCORES = 8
SEG_PER_CORE = NSEG // NCORES            # 64
TOK = L // P                             # 256 tokens per chunk
F32 = mybir.dt.float32
F16 = mybir.dt.float16
I32 = mybir.dt.int32

# cf16 layout (f16): [0:256] refine pos encode, [256] min-chunk encode
# (128-p), [257] max-chunk encode (p+1), [258] chunk index p, [259] pad
CFH_W = 260
# cf32 layout (f32): [0] side sign, [1] global decode const, [2] pad,
# [3] segment id, [4] word id 0, [5] word id 1
CFS_W = 6


def build_nc():
    nc = bacc.Bacc("TRN2", target_bir_lowering=False, debug=False)

    x = nc.dram_tensor("x", [L, H], F32, kind="ExternalInput")
    idpairs = nc.dram_tensor("idpairs", [P, TOK, 2], I32, kind="ExternalInput")
    cf16 = nc.dram_tensor("cf16", [P, CFH_W], F16, kind="ExternalInput")
    cf32 = nc.dram_tensor("cf32", [P, CFS_W], F32, kind="ExternalInput")
    ci32 = nc.dram_tensor("ci32", [P, SEG_PER_CORE], I32, kind="ExternalInput")
    out = nc.dram_tensor("out", [SEG_PER_CORE, 2 * H], F32, kind="ExternalOutput")

    A = mybir.AluOpType

    with tile.TileContext(nc) as tc:
        with tc.tile_pool(name="sb", bufs=1) as sb, \
             tc.tile_pool(name="ps", bufs=1, space="PSUM") as ps:

            # warm up the gpsimd partition-reduce microcode FIRST (its ~6.5us
            # one-time code load overlaps the input DMAs + main pass)
            warm = sb.tile([P, 2], F16)
            nc.gpsimd.memset(warm[:], 0.0)
            nc.gpsimd.partition_all_reduce(warm[:], warm[:], channels=P,
                                           reduce_op=bass_isa.ReduceOp.max)

            # ---- loads (sync/scalar queues; gpsimd is busy loading ucode) ----
            idp = sb.tile([P, TOK, 2], I32)
            nc.sync.dma_start(idp[:, 0:TOK // 2, :],
                              idpairs.ap()[:, 0:TOK // 2, :])
            nc.scalar.dma_start(idp[:, TOK // 2:TOK, :],
                                idpairs.ap()[:, TOK // 2:TOK, :])
            cfh = sb.tile([P, CFH_W], F16)
            nc.sync.dma_start(cfh[:], cf16.ap())
            cfs = sb.tile([P, CFS_W], F32)
            nc.scalar.dma_start(cfs[:], cf32.ap())
            cis = sb.tile([P, SEG_PER_CORE], I32)
            nc.sync.dma_start(cis[:], ci32.ap())
            # f16 ids + chunk-index column for the PE gather (scalar engine,
            # off the DVE critical path)
            idsf = sb.tile([P, TOK], F16)
            nc.scalar.copy(idsf[:], idp[:, :, 0])

            # ---- main pass: bit-packed presence per (chunk, segment) ----
            lo5 = sb.tile([P, TOK], I32)
            nc.vector.tensor_scalar(lo5[:], idp[:, :, 0], 31, None,
                                    op0=A.bitwise_and)
            hi4 = sb.tile([P, TOK], I32)
            nc.vector.tensor_scalar(hi4[:], idp[:, :, 0], 5, None,
                                    op0=A.arith_shift_right)
            cand = sb.tile([P, 2, TOK], I32)
            eq0 = sb.tile([P, TOK], I32)
            nc.vector.tensor_scalar(eq0[:], hi4[:], cfs[:, 4:5], None,
                                    op0=A.is_equal)
            nc.vector.tensor_tensor(out=cand[:, 0], in0=eq0[:], in1=lo5[:],
                                    op=A.logical_shift_left)
            eq1 = sb.tile([P, TOK], I32)
            nc.vector.tensor_scalar(eq1[:], hi4[:], cfs[:, 5:6], None,
                                    op0=A.is_equal)
            nc.vector.tensor_tensor(out=cand[:, 1], in0=eq1[:], in1=lo5[:],
                                    op=A.logical_shift_left)
            # bitwise-OR tree over the token axis: 256 -> 1 per word
            lv = cand
            width = TOK
            while width > 1:
                half = width // 2
                nxt = sb.tile([P, 2, half], I32, tag=f"or{half}")
                nc.vector.tensor_tensor(out=nxt[:], in0=lv[:, :, 0:half],
                                        in1=lv[:, :, half:width],
                                        op=A.bitwise_or)
                lv = nxt
                width = half
            words = lv                                    # [P, 2, 1]

            # ---- decode: first/last chunk per segment ----
            bits_in = words[:, :, 0].unsqueeze(2).broadcast_to([P, 2, 32])
            cis_v = cis[:, 0:SEG_PER_CORE].rearrange("p (a b) -> p a b", a=2)
            andm = sb.tile([P, 2, 32], I32)
            nc.vector.tensor_tensor(out=andm[:], in0=bits_in, in1=cis_v,
                                    op=A.bitwise_and)
            andf = andm[:].rearrange("p a b -> p (a b)")
            enc = sb.tile([P, P], F16)
            nc.vector.scalar_tensor_tensor(
                out=enc[:, 0:SEG_PER_CORE], in0=andf, scalar=0,
                in1=cfh[:, 256:257].broadcast_to([P, SEG_PER_CORE]),
                op0=A.not_equal, op1=A.mult)
            nc.vector.scalar_tensor_tensor(
                out=enc[:, SEG_PER_CORE:P], in0=andf, scalar=0,
                in1=cfh[:, 257:258].broadcast_to([P, SEG_PER_CORE]),
                op0=A.not_equal, op1=A.mult)
            # max over chunks, replicated to every partition (gpsimd)
            valbc = sb.tile([P, P], F16)
            nc.gpsimd.partition_all_reduce(valbc[:], enc[:], channels=P,
                                           reduce_op=bass_isa.ReduceOp.max)
            # argmax chunk one-hot (encodes are distinct per chunk)
            onehot = sb.tile([P, P], F16)
            nc.vector.tensor_tensor(out=onehot[:], in0=enc[:], in1=valbc[:],
                                    op=A.is_equal)

            # ---- on-chip gather of candidate chunks' ids via PE ----
            grows = ps.tile([P, TOK], F32)
            nc.tensor.matmul(grows[:], onehot[:], idsf[:],
                             start=True, stop=True)
            gsel = ps.tile([P, 1], F32)
            nc.tensor.matmul(gsel[:], onehot[:], cfh[:, 258:259],
                             start=True, stop=True)

            # ---- refine: exact within-chunk position ----
            encr = sb.tile([P, TOK], F16)
            nc.vector.scalar_tensor_tensor(
                out=encr[:], in0=grows[:], scalar=cfs[:, 3:4],
                in1=cfh[:, 0:TOK], op0=A.is_equal, op1=A.mult)
            val2 = sb.tile([P, 1], F32)
            nc.vector.tensor_reduce(val2[:], encr[:],
                                    axis=mybir.AxisListType.X,
                                    op=A.max)

            # ---- global row index: clamp(256*chunk + sgn*val2 + cst) ----
            g = sb.tile([P, 1], F32)
            nc.vector.tensor_scalar(g[:], gsel[:], float(TOK), None,
                                    op0=A.mult)
            gt = sb.tile([P, 1], F32)
            nc.vector.scalar_tensor_tensor(
                out=gt[:], in0=val2[:], scalar=cfs[:, 0:1], in1=g[:],
                op0=A.mult, op1=A.add)
            nc.vector.tensor_scalar(gt[:], gt[:], cfs[:, 1:2], None,
                                    op0=A.add)
            nc.vector.tensor_scalar(gt[:], gt[:], float(L - 1), 0.0,
                                    op0=A.min, op1=A.max)
            gi = sb.tile([P, 1], I32)
            nc.vector.tensor_copy(gi[:], gt[:])
            gi2 = sb.tile([P, 1], I32)
            nc.vector.tensor_scalar(gi2[:], gi[:], 1, None,
                                    op0=A.logical_shift_left)

            # ---- gather rows (half-row split, pipelined with writes) ----
            # x viewed as [2L, H/2] half-rows so the offset coefficient
            # (derived from the view shape) matches the row stride
            HH = H // 2
            xv = x.ap().rearrange("r (s h) -> (r s) h", s=2)
            rows = sb.tile([P, H], F32)
            nc.gpsimd.indirect_dma_start(
                out=rows[:, 0:HH], out_offset=None, in_=xv,
                in_offset=bass.IndirectOffsetOnAxis(ap=gi2[:, 0:1], axis=0))
            nc.sync.dma_start(out.ap()[:, 0:HH], rows[0:SEG_PER_CORE, 0:HH])
            nc.scalar.dma_start(out.ap()[:, H:H + HH],
                                rows[SEG_PER_CORE:P, 0:HH])
            nc.gpsimd.indirect_dma_start(
                out=rows[:, HH:H], out_offset=None, in_=xv,
                in_offset=bass.IndirectOffsetOnAxis(ap=gi2[:, 0:1], axis=0),
                element_offset=HH)
            nc.sync.dma_start(out.ap()[:, HH:H], rows[0:SEG_PER_CORE, HH:H])
            nc.scalar.dma_start(out.ap()[:, H + HH:2 * H],
                                rows[SEG_PER_CORE:P, HH:H])

    nc.compile()
    return nc


_NC = None


def _get_nc():
    global _NC
    if _NC is None:
        _NC = build_nc()
    return _NC


def make_in_maps(input, number_mask):
    x = np.ascontiguousarray(np.asarray(input), dtype=np.float32).reshape(L, H)
    nm = np.ascontiguousarray(np.asarray(number_mask))
    if nm.dtype != np.int64:
        nm = nm.astype(np.int64)
    idpairs = nm.reshape(L).view(np.int32).reshape(P, TOK, 2)

    r = np.arange(P)
    side_max = r >= SEG_PER_CORE                  # rows 0-63 min, 64-127 max
    t = np.arange(TOK, dtype=np.float16)
    cf16 = np.zeros((P, CFH_W), dtype=np.float16)
    cf16[:SEG_PER_CORE, 0:TOK] = TOK - t          # refine encode, min side
    cf16[SEG_PER_CORE:, 0:TOK] = t + 1            # refine encode, max side
    cf16[:, 256] = (P - r).astype(np.float16)     # min-chunk encode coeff
    cf16[:, 257] = (r + 1).astype(np.float16)     # max-chunk encode coeff
    cf16[:, 258] = r.astype(np.float16)           # chunk index

    maskbits = np.tile((np.int32(1) << (np.arange(SEG_PER_CORE, dtype=np.int32)
                                        % 32)), (P, 1)).astype(np.int32)

    in_maps = []
    for c in range(NCORES):
        cf32 = np.zeros((P, CFS_W), dtype=np.float32)
        cf32[:, 0] = np.where(side_max, 1.0, -1.0)          # sgn
        cf32[:, 1] = np.where(side_max, -1.0, float(TOK))   # cst2
        cf32[:, 3] = c * SEG_PER_CORE + (r % SEG_PER_CORE)  # segment id
        cf32[:, 4] = 2 * c                                  # word id 0
        cf32[:, 5] = 2 * c + 1                              # word id 1
        in_maps.append({"x": x, "idpairs": idpairs, "cf16": cf16,
                        "cf32": cf32, "ci32": maskbits})
    return in_maps


def kernel(input, number_mask, n, concat, **_):
    assert int(n) == NSEG and int(concat) == 1
    nc = _get_nc()
    in_maps = make_in_maps(input, number_mask)
    res = bass_utils.run_bass_kernel_spmd(nc, in_maps, core_ids=list(range(NCORES)))
    return np.concatenate([res.results[c]["out"] for c in range(NCORES)], axis=0)


# revision 15
# speedup vs baseline: 1.2066x; 1.0189x over previous
"""Trainium2 Bass kernel for nn_AwareDecoder segment first/last gather.

Problem: input [16, 2048, 1024] f32, number_mask [16, 2048] int64 with ids in
[0, 512]. For each segment id i in [0, 512): find first/last row-major token
position with that id, gather those rows of the flattened input, concat ->
out [512, 2048] f32.

Strategy (8 NeuronCores, segment-sharded - no collectives):
  core c owns segments [64c, 64c+64). Each core:
    - DMAs the (tiny, 256KB) id array as int32 (lo,hi) pairs; chunk p =
      tokens [256p, 256p+256) lives on partition p,
    - bit-packs per-chunk presence of its 64 segments into 2 int32 words per
      chunk (fused eq-compare on the id high bits + variable left-shift by
      the id low bits, then a bitwise-OR tree over the 256 tokens),
    - decodes first/last chunk per segment: bit-test, position encode, then
      a gpsimd partition_all_reduce(max) which also broadcasts, so the
      argmax chunk one-hot is a single eq-compare (no transposes),
    - gathers the candidate chunks' ids ON-CHIP with a one-hot PE matmul
      that also emits the selected chunk index as an extra column, then
      finds the exact within-chunk position with one fused
      scalar_tensor_tensor + max-reduce,
    - turns (chunk, pos) into global row indices and pulls its 64 first +
      64 last rows (4KB each) straight from HBM with two pipelined hardware
      indirect DMAs (reads only 512KB of the 128MB input), overlapping the
      output writeback of the first half with the second gather.
Host concatenates the 8 slices.
"""
import numpy as np

import concourse.bass as bass
import concourse.tile as tile
from concourse import bacc, bass_isa, mybir
from concourse import bass_utils

P = 128            # partitions / token chunks
L = 32768          # B*S tokens
H = 1024           # hidden
NSEG = 512         # segments
NCORES = 8
SEG_PER_CORE = NSEG // NCORES            # 64
TOK = L // P                             # 256 tokens per chunk
F32 = mybir.dt.float32
F16 = mybir.dt.float16
I32 = mybir.dt.int32

# cf16 layout (f16): [0:256] refine pos encode, [256] min-chunk encode
# (128-p), [257] max-chunk encode (p+1), [258] chunk index p, [259] pad
CFH_W = 260
# cf32 layout (f32): [0] side sign, [1] global decode const, [2] pad,
# [3] segment id, [4] word id 0, [5] word id 1
CFS_W = 6


def build_nc():
    nc = bacc.Bacc("TRN2", target_bir_lowering=False, debug=False)

    x = nc.dram_tensor("x", [L, H], F32, kind="ExternalInput")
    idpairs = nc.dram_tensor("idpairs", [P, TOK, 2], I32, kind="ExternalInput")
    cf16 = nc.dram_tensor("cf16", [P, CFH_W], F16, kind="ExternalInput")
    cf32 = nc.dram_tensor("cf32", [P, CFS_W], F32, kind="ExternalInput")
    ci32 = nc.dram_tensor("ci32", [P, SEG_PER_CORE], I32, kind="ExternalInput")
    out = nc.dram_tensor("out", [SEG_PER_CORE, 2 * H], F32, kind="ExternalOutput")

    A = mybir.AluOpType

    with tile.TileContext(nc) as tc:
        with tc.tile_pool(name="sb", bufs=1) as sb, \
             tc.tile_pool(name="ps", bufs=1, space="PSUM") as ps:

            # warm up the gpsimd partition-reduce microcode FIRST (its ~6.5us
            # one-time code load overlaps the input DMAs + main pass)
            warm = sb.tile([P, 2], F16)
            nc.gpsimd.memset(warm[:], 0.0)
            nc.gpsimd.partition_all_reduce(warm[:], warm[:], channels=P,
                                           reduce_op=bass_isa.ReduceOp.max)

            # ---- loads: idpairs quartered across 4 queues (gpsimd is busy
            # loading ucode); consts ordered by first use ----
            idp = sb.tile([P, TOK, 2], I32)
            nc.sync.dma_start(idp[:, 0:TOK // 2, :],
                              idpairs.ap()[:, 0:TOK // 2, :])
            nc.scalar.dma_start(idp[:, TOK // 2:TOK, :],
                                idpairs.ap()[:, TOK // 2:TOK, :])
            cfs = sb.tile([P, CFS_W], F32)
            nc.scalar.dma_start(cfs[:], cf32.ap())
            cis = sb.tile([P, SEG_PER_CORE], I32)
            nc.sync.dma_start(cis[:], ci32.ap())
            cfh = sb.tile([P, CFH_W], F16)
            nc.sync.dma_start(cfh[:], cf16.ap())
            # f16 ids + chunk-index column for the PE gather (scalar engine,
            # off the DVE critical path)
            idsf = sb.tile([P, TOK], F16)
            nc.scalar.copy(idsf[:], idp[:, :, 0])

            # ---- main pass: bit-packed presence per (chunk, segment) ----
            lo5 = sb.tile([P, TOK], I32)
            nc.vector.tensor_scalar(lo5[:], idp[:, :, 0], 31, None,
                                    op0=A.bitwise_and)
            hi4 = sb.tile([P, TOK], I32)
            nc.vector.tensor_scalar(hi4[:], idp[:, :, 0], 5, None,
                                    op0=A.arith_shift_right)
            cand = sb.tile([P, 2, TOK], I32)
            eq0 = sb.tile([P, TOK], I32)
            nc.vector.tensor_scalar(eq0[:], hi4[:], cfs[:, 4:5], None,
                                    op0=A.is_equal)
            nc.vector.tensor_tensor(out=cand[:, 0], in0=eq0[:], in1=lo5[:],
                                    op=A.logical_shift_left)
            eq1 = sb.tile([P, TOK], I32)
            nc.vector.tensor_scalar(eq1[:], hi4[:], cfs[:, 5:6], None,
                                    op0=A.is_equal)
            nc.vector.tensor_tensor(out=cand[:, 1], in0=eq1[:], in1=lo5[:],
                                    op=A.logical_shift_left)
            # bitwise-OR tree over the token axis: 256 -> 1 per word
            lv = cand
            width = TOK
            while width > 1:
                half = width // 2
                nxt = sb.tile([P, 2, half], I32, tag=f"or{half}")
                nc.vector.tensor_tensor(out=nxt[:], in0=lv[:, :, 0:half],
                                        in1=lv[:, :, half:width],
                                        op=A.bitwise_or)
                lv = nxt
                width = half
            words = lv                                    # [P, 2, 1]

            # ---- decode: first/last chunk per segment ----
            bits_in = words[:, :, 0].unsqueeze(2).broadcast_to([P, 2, 32])
            cis_v = cis[:, 0:SEG_PER_CORE].rearrange("p (a b) -> p a b", a=2)
            andm = sb.tile([P, 2, 32], I32)
            nc.vector.tensor_tensor(out=andm[:], in0=bits_in, in1=cis_v,
                                    op=A.bitwise_and)
            andf = andm[:].rearrange("p a b -> p (a b)")
            enc = sb.tile([P, P], F16)
            nc.vector.scalar_tensor_tensor(
                out=enc[:, 0:SEG_PER_CORE], in0=andf, scalar=0,
                in1=cfh[:, 256:257].broadcast_to([P, SEG_PER_CORE]),
                op0=A.not_equal, op1=A.mult)
            nc.vector.scalar_tensor_tensor(
                out=enc[:, SEG_PER_CORE:P], in0=andf, scalar=0,
                in1=cfh[:, 257:258].broadcast_to([P, SEG_PER_CORE]),
                op0=A.not_equal, op1=A.mult)
            # max over chunks, replicated to every partition (gpsimd)
            valbc = sb.tile([P, P], F16)
            nc.gpsimd.partition_all_reduce(valbc[:], enc[:], channels=P,
                                           reduce_op=bass_isa.ReduceOp.max)
            # argmax chunk one-hot (encodes are distinct per chunk)
            onehot = sb.tile([P, P], F16)
            nc.vector.tensor_tensor(out=onehot[:], in0=enc[:], in1=valbc[:],
                                    op=A.is_equal)

            # ---- on-chip gather of candidate chunks' ids via PE ----
            grows = ps.tile([P, TOK], F32)
            nc.tensor.matmul(grows[:], onehot[:], idsf[:],
                             start=True, stop=True)
            gsel = ps.tile([P, 1], F32)
            nc.tensor.matmul(gsel[:], onehot[:], cfh[:, 258:259],
                             start=True, stop=True)

            # ---- refine: exact within-chunk position ----
            encr = sb.tile([P, TOK], F16)
            nc.vector.scalar_tensor_tensor(
                out=encr[:], in0=grows[:], scalar=cfs[:, 3:4],
                in1=cfh[:, 0:TOK], op0=A.is_equal, op1=A.mult)
            val2 = sb.tile([P, 1], F32)
            nc.vector.tensor_reduce(val2[:], encr[:],
                                    axis=mybir.AxisListType.X,
                                    op=A.max)

            # ---- half-row index: clamp(512*chunk + 2*sgn*val2 + 2*cst) ----
            g = sb.tile([P, 1], F32)
            nc.vector.tensor_scalar(g[:], gsel[:], float(2 * TOK), None,
                                    op0=A.mult)
            gt = sb.tile([P, 1], F32)
            nc.vector.scalar_tensor_tensor(
                out=gt[:], in0=val2[:], scalar=cfs[:, 0:1], in1=g[:],
                op0=A.mult, op1=A.add)
            nc.vector.tensor_scalar(gt[:], gt[:], cfs[:, 1:2],
                                    float(2 * L - 2),
                                    op0=A.add, op1=A.min)
            gi2 = sb.tile([P, 1], I32)
            nc.vector.tensor_copy(gi2[:], gt[:])

            # ---- gather rows (half-row split, pipelined with writes) ----
            # x viewed as [2L, H/2] half-rows so the offset coefficient
            # (derived from the view shape) matches the row stride
            HH = H // 2
            xv = x.ap().rearrange("r (s h) -> (r s) h", s=2)
            rows = sb.tile([P, H], F32)
            nc.gpsimd.indirect_dma_start(
                out=rows[:, 0:HH], out_offset=None, in_=xv,
                in_offset=bass.IndirectOffsetOnAxis(ap=gi2[:, 0:1], axis=0))
            nc.sync.dma_start(out.ap()[:, 0:HH], rows[0:SEG_PER_CORE, 0:HH])
            nc.scalar.dma_start(out.ap()[:, H:H + HH],
                                rows[SEG_PER_CORE:P, 0:HH])
            nc.gpsimd.indirect_dma_start(
                out=rows[:, HH:H], out_offset=None, in_=xv,
                in_offset=bass.IndirectOffsetOnAxis(ap=gi2[:, 0:1], axis=0),
                element_offset=HH)
            nc.sync.dma_start(out.ap()[:, HH:H], rows[0:SEG_PER_CORE, HH:H])
            nc.scalar.dma_start(out.ap()[:, H + HH:2 * H],
                                rows[SEG_PER_CORE:P, HH:H])

    nc.compile()
    return nc


_NC = None


def _get_nc():
    global _NC
    if _NC is None:
        _NC = build_nc()
    return _NC


def make_in_maps(input, number_mask):
    x = np.ascontiguousarray(np.asarray(input), dtype=np.float32).reshape(L, H)
    nm = np.ascontiguousarray(np.asarray(number_mask))
    if nm.dtype != np.int64:
        nm = nm.astype(np.int64)
    idpairs = nm.reshape(L).view(np.int32).reshape(P, TOK, 2)

    r = np.arange(P)
    side_max = r >= SEG_PER_CORE                  # rows 0-63 min, 64-127 max
    t = np.arange(TOK, dtype=np.float16)
    cf16 = np.zeros((P, CFH_W), dtype=np.float16)
    cf16[:SEG_PER_CORE, 0:TOK] = TOK - t          # refine encode, min side
    cf16[SEG_PER_CORE:, 0:TOK] = t + 1            # refine encode, max side
    cf16[:, 256] = (P - r).astype(np.float16)     # min-chunk encode coeff
    cf16[:, 257] = (r + 1).astype(np.float16)     # max-chunk encode coeff
    cf16[:, 258] = r.astype(np.float16)           # chunk index

    maskbits = np.tile((np.int32(1) << (np.arange(SEG_PER_CORE, dtype=np.int32)
                                        % 32)), (P, 1)).astype(np.int32)

    in_maps = []
    for c in range(NCORES):
        cf32 = np.zeros((P, CFS_W), dtype=np.float32)
        cf32[:, 0] = np.where(side_max, 2.0, -2.0)            # 2*sgn
        cf32[:, 1] = np.where(side_max, -2.0, float(2 * TOK))  # 2*cst2
        cf32[:, 3] = c * SEG_PER_CORE + (r % SEG_PER_CORE)  # segment id
        cf32[:, 4] = 2 * c                                  # word id 0
        cf32[:, 5] = 2 * c + 1                              # word id 1
        in_maps.append({"x": x, "idpairs": idpairs, "cf16": cf16,
                        "cf32": cf32, "ci32": maskbits})
    return in_maps


def kernel(input, number_mask, n, concat, **_):
    assert int(n) == NSEG and int(concat) == 1
    nc = _get_nc()
    in_maps = make_in_maps(input, number_mask)
    res = bass_utils.run_bass_kernel_spmd(nc, in_maps, core_ids=list(range(NCORES)))
    return np.concatenate([res.results[c]["out"] for c in range(NCORES)], axis=0)
